# revision 5
# baseline (speedup 1.0000x reference)
"""Trainium2 Bass kernel for AgnosticNonlinearInteractionBlock (GNN message passing).

v4 — Sharding: edges partitioned by receiver node range across 8 cores; each
core computes full output rows for its 1250-node slice. No collectives.

Key design points vs the v1 baseline:
  - Balanced receiver blocks (host greedy bin-pack by in-degree) -> T_BLK=16
    (was 18): 160 edge tiles/core instead of 180, ~2% edge padding.
  - All per-edge y scalars folded into the DMA'd scatter matrices
    Sq = [S1*y0 | S1*y1_0 | S1*y1_1 | S1*y1_2]; the DVE CG product is 4 pure
    tensor_tensor multiplies (two flat SBUF 2x ops, two psum-direct
    broadcast ops) — no scalar_tensor_tensor, no per-edge y reads.
  - msg-stationary scatter: macc psum is [feature, node] per block; no PE
    transposes, no mid-linear transpose copies. PSUM zero-region hazard
    (start=True pends the whole 2KB bank) handled by one full-bank zeroing
    matmul per bank per block + start=False accumulation.
  - MLP / w4 matmuls packed in concurrent disjoint PE row/col groups.
  - Whole-block MLP runs before the per-tile chain so the scalar FIFO is not
    head-of-line blocked; phase-1 stage copies alternate scalar/vector.
  - osb stored [c_out, x, node] so phase-3 skip_tp Z-multiplies are flat
    256-col ops, split between vector and gpsimd engines.
  - macc double-buffered in PSUM (psA 512-wide x2, psB 2x2 banks, psH 2).
"""

import sys

sys.path.insert(0, "/opt/trn_rl_repo")

import numpy as np
import ml_dtypes

BF16 = ml_dtypes.bfloat16

# Problem constants (hardcoded per contest contract)
N, E = 10000, 160000
C, A, R, H = 128, 10, 8, 64
AVG_NEI = 16.0
INV_SQRT3 = 1.0 / np.sqrt(3.0)

NCORES = 8
NPC = N // NCORES          # nodes per core = 1250
BLOCKS = 10                # 128-node blocks per core (1280 padded)
LNPAD = BLOCKS * 128       # 1280 local padded nodes
T_BLK = 16                 # edge tiles (x128) per block (balanced blocks)
E_BLK = T_BLK * 128        # 2048 edges per block
E_CAP = BLOCKS * E_BLK     # 20480 edges per core
TILES = BLOCKS * T_BLK     # 160 tiles per core
N_PAD = 10112              # 79 * 128 padded node count for up-table
NT_UP = N_PAD // 128       # 79 node tiles for linear_up
MLP_CH = 512               # MLP chunk size (edges): 2 chunks of 1024 per block


def _balance_blocks(deg):
    """Greedy bin-pack 1250 nodes into BLOCKS blocks of <=128 by in-degree.
    Returns lperm [LNPAD]: padded position -> local node id (-1 = pad)."""
    order = np.argsort(-deg, kind="stable")
    loads = np.zeros(BLOCKS, np.int64)
    sizes = np.zeros(BLOCKS, np.int64)
    members = [[] for _ in range(BLOCKS)]
    for n in order:
        cand = min((b for b in range(BLOCKS) if sizes[b] < 128),
                   key=lambda b: loads[b])
        members[cand].append(n)
        loads[cand] += deg[n]
        sizes[cand] += 1
    assert loads.max() <= E_BLK, f"block overflow: {loads.max()} > {E_BLK}"
    lperm = np.full(LNPAD, -1, np.int64)
    for b in range(BLOCKS):
        m = np.asarray(members[b], np.int64)
        lperm[b * 128: b * 128 + len(m)] = m
    return lperm


def _prep_host(node_attrs, node_feats, edge_attrs, edge_feats, edge_index,
               W_up0, W_up1, W_mlp1, W_mlp2, W_mlp3, W_mlp4,
               W_lin0, W_lin1, W_skip0, W_skip1):
    """Build per-core input arrays (marshalling only, no NN math)."""
    send = np.asarray(edge_index[0]).astype(np.int64)
    recv = np.asarray(edge_index[1]).astype(np.int64)
    ef = np.asarray(edge_feats, dtype=np.float32)
    ea = np.asarray(edge_attrs, dtype=np.float32)

    # ---- shared (replicated) weight arrays, scales folded in ----
    w1 = (np.asarray(W_mlp1, np.float32) / np.sqrt(R)).astype(BF16)          # [8,64]
    w2 = (np.asarray(W_mlp2, np.float32) / np.sqrt(H)).astype(BF16)          # [64,64]
    w3 = (np.asarray(W_mlp3, np.float32) / np.sqrt(H)).astype(BF16)          # [64,64]
    w4 = np.asarray(W_mlp4, np.float32) / np.sqrt(H)                          # [64,512]
    ws1, ws2, wv1, wv2 = w4[:, 0:C], w4[:, C:2*C], w4[:, 2*C:3*C], w4[:, 3*C:4*C]
    ws2 = ws2 * INV_SQRT3              # fold w3j coefficient
    # reordered layout: [ws1 | wv1 | wv2 | ws2]  -> [64, 512]
    w4e = np.concatenate([ws1, wv1, wv2, ws2], axis=1).astype(BF16)
    wup = np.stack([np.asarray(W_up0, np.float32), np.asarray(W_up1, np.float32)]
                   ).astype(np.float32) / np.sqrt(C)
    wup = wup.astype(BF16)                                                    # [2,128,128]
    norm = np.sqrt(2 * C) * AVG_NEI
    wl0 = (np.asarray(W_lin0, np.float32) / norm).astype(BF16)                # [256,128]
    wl1 = (np.asarray(W_lin1, np.float32) / norm).astype(BF16)
    # wlin chunks: [4,128,128] = [l0_c0, l0_c1, l1_c0, l1_c1]
    wlin = np.stack([wl0[:C], wl0[C:], wl1[:C], wl1[C:]]).astype(BF16)
    fan = np.sqrt(C * A)
    wsk0 = np.asarray(W_skip0, np.float32) / fan                              # [C,A,C]
    wsk1 = np.asarray(W_skip1, np.float32) / fan
    # wsk chunks: [2,10,128,128]  (x in {s, v}, a)
    wsk = np.stack([wsk0.transpose(1, 0, 2), wsk1.transpose(1, 0, 2)]).astype(BF16)

    # node_feats transposed planes [4,128,N_PAD]: s, v0, v1, v2 component-major
    nfT = np.zeros((4, 128, N_PAD), np.float32)
    nfT[0, :, :N] = np.asarray(node_feats, np.float32)[:, :C].T
    v = np.asarray(node_feats, np.float32)[:, C:].reshape(N, C, 3)
    for i in range(3):
        nfT[1 + i, :, :N] = v[:, :, i].T
    nfT = nfT.reshape(512, N_PAD).astype(BF16)

    attrs = np.asarray(node_attrs, np.float32)
    deg_all = np.bincount(recv, minlength=N)

    in_maps = []
    lperms = []
    for m in range(NCORES):
        lo, hi = m * NPC, (m + 1) * NPC
        lperm = _balance_blocks(deg_all[lo:hi])      # padded pos -> local node
        lperms.append(lperm)
        inv = np.full(NPC, -1, np.int64)             # local node -> padded pos
        real_pos = lperm >= 0
        inv[lperm[real_pos]] = np.nonzero(real_pos)[0]

        mask = (recv >= lo) & (recv < hi)
        eidx = np.nonzero(mask)[0]
        pos = inv[recv[eidx] - lo]                   # padded position [0,1280)
        blk = pos // 128
        # bucket edges by block, pad each block to E_BLK
        perm = np.full(E_CAP, -1, np.int64)          # -1 = dummy edge
        slot = np.zeros(E_CAP, np.int64)             # slot within block [0,128)
        for b in range(BLOCKS):
            sel = blk == b
            be = eidx[sel]
            assert len(be) <= E_BLK, f"core {m} block {b}: {len(be)} > {E_BLK}"
            bs = pos[sel] % 128
            o = np.argsort(send[be], kind="stable")  # DRAM page locality
            perm[b * E_BLK: b * E_BLK + len(be)] = be[o]
            slot[b * E_BLK: b * E_BLK + len(be)] = bs[o]
        real = perm >= 0
        psafe = np.where(real, perm, 0)

        # edge feats transposed [8, E_CAP], zeros for dummies
        efT = np.where(real[None, :], ef[psafe].T, 0.0).astype(BF16)
        # sender indices wrapped into 16 partitions, replicated to 128
        snd = np.where(real, send[psafe], 0).astype(np.int16)
        sidx = np.zeros((128, TILES * 8), np.int16)  # per block: [128, E_BLK//16]
        for b in range(BLOCKS):
            s_b = snd[b * E_BLK: (b + 1) * E_BLK]
            w = s_b.reshape(E_BLK // 16, 16).T                 # [16, 128]
            sidx[:, b * (E_BLK // 16): (b + 1) * (E_BLK // 16)] = np.tile(w, (8, 1))
        # scatter one-hot variants [E_CAP, 4*128]: [S*y0 | S*y10 | S*y11 | S*y12]
        ya = np.where(real[:, None], ea[psafe], 0.0)           # [E_CAP, 4]
        S1 = np.zeros((E_CAP, 128), np.float32)
        S1[np.arange(E_CAP), slot] = 1.0
        S1[~real] = 0.0
        Sq = np.concatenate([S1 * ya[:, i:i + 1] for i in range(4)],
                            axis=1).astype(BF16)
        # replicated node attrs [128, 10*1280] in PERMUTED node order
        pattr = np.zeros((LNPAD, A), np.float32)
        pattr[real_pos] = attrs[lo:hi][lperm[real_pos]]
        arep = np.broadcast_to(pattr.T[:, None, :], (A, 128, LNPAD))
        arep = arep.transpose(1, 0, 2).reshape(128, A * LNPAD).astype(BF16)

        in_maps.append(dict(
            efT=efT, sidx=sidx, Sq=Sq, arep=arep, nfT=nfT,
            w1=w1, w2=w2, w3=w3, w4e=w4e,
            wup=wup.reshape(256, 128), wlin=wlin.reshape(512, 128),
            wsk=wsk.reshape(2560, 128),
        ))
    _prep_host.lperms = lperms
    return in_maps


def _assemble_output(results):
    """results: per-core dict with 'out' [512, 1280] f32 -> full [N, 512] f32."""
    out = np.zeros((N, 4 * C), np.float32)
    lperms = _prep_host.lperms
    for m in range(NCORES):
        o = np.asarray(results[m]["out"], np.float32)          # [512, 1280]
        lperm = lperms[m]
        real = lperm >= 0
        gl = m * NPC + lperm[real]                              # global node ids
        out[gl, :C] = o[:C, real].T                             # o_s
        for i in range(3):
            out[gl, C + i::3] = o[C * (1 + i):C * (2 + i), real].T
    return out


# ---------------------------------------------------------------------------
# Device kernel builder
# ---------------------------------------------------------------------------

_CACHE = {}


def _build_nc():
    import os
    from concourse import bass, bacc, tile, mybir

    dt = mybir.dt
    AF = mybir.ActivationFunctionType
    OP = mybir.AluOpType
    ITERS = int(os.environ.get("KITERS", "1"))
    PHASES = os.environ.get("PHASES", "full")

    nc = bacc.Bacc("TRN2", target_bir_lowering=False, debug=False,
                   num_devices=NCORES, num_swdge_queues=4)

    # DRAM I/O
    d_efT = nc.dram_tensor("efT", [8, E_CAP], dt.bfloat16, kind="ExternalInput")
    d_sidx = nc.dram_tensor("sidx", [128, TILES * 8], dt.int16, kind="ExternalInput")
    d_Sq = nc.dram_tensor("Sq", [E_CAP, 512], dt.bfloat16, kind="ExternalInput")
    d_arep = nc.dram_tensor("arep", [128, A * LNPAD], dt.bfloat16, kind="ExternalInput")
    d_nfT = nc.dram_tensor("nfT", [512, N_PAD], dt.bfloat16, kind="ExternalInput")
    d_w1 = nc.dram_tensor("w1", [8, 64], dt.bfloat16, kind="ExternalInput")
    d_w2 = nc.dram_tensor("w2", [64, 64], dt.bfloat16, kind="ExternalInput")
    d_w3 = nc.dram_tensor("w3", [64, 64], dt.bfloat16, kind="ExternalInput")
    d_w4e = nc.dram_tensor("w4e", [64, 512], dt.bfloat16, kind="ExternalInput")
    d_wup = nc.dram_tensor("wup", [256, 128], dt.bfloat16, kind="ExternalInput")
    d_wlin = nc.dram_tensor("wlin", [512, 128], dt.bfloat16, kind="ExternalInput")
    d_wsk = nc.dram_tensor("wsk", [2560, 128], dt.bfloat16, kind="ExternalInput")
    d_out = nc.dram_tensor("out", [512, LNPAD], dt.float32, kind="ExternalOutput")
    d_tables = [nc.dram_tensor(f"table{k}", [N_PAD, 512], dt.bfloat16,
                               kind="Internal") for k in range(2)]

    with tile.TileContext(nc) as tc:
        with (
            tc.tile_pool(name="const", bufs=1) as cpool,
            tc.tile_pool(name="work", bufs=4) as wpool,
            tc.tile_pool(name="gbuf", bufs=2) as gpool,
            tc.tile_pool(name="spool", bufs=2) as spool,
            tc.tile_pool(name="upool", bufs=5) as upool,
            tc.tile_pool(name="msg", bufs=4) as mpool,
            tc.tile_pool(name="blk", bufs=2) as bpool,
            tc.tile_pool(name="psA", bufs=2, space=bass.MemorySpace.PSUM) as psA,
            tc.tile_pool(name="psH", bufs=2, space=bass.MemorySpace.PSUM) as psH,
            tc.tile_pool(name="psB", bufs=2, space=bass.MemorySpace.PSUM) as psB,
        ):
            # ---- resident constants ----
            sidx = cpool.tile([128, TILES * 8], dt.int16)
            nc.sync.dma_start(sidx[:], d_sidx[:])
            arep = cpool.tile([128, A * LNPAD], dt.bfloat16)
            nc.sync.dma_start(arep[:], d_arep[:])
            w1 = cpool.tile([72, 64], dt.bfloat16)
            nc.sync.dma_start(w1[0:8, :], d_w1[:])
            nc.sync.dma_start(w1[64:72, :], d_w1[:])
            w2 = cpool.tile([128, 64], dt.bfloat16)
            nc.sync.dma_start(w2[0:64, :], d_w2[:])
            nc.sync.dma_start(w2[64:128, :], d_w2[:])
            w3 = cpool.tile([128, 64], dt.bfloat16)
            nc.sync.dma_start(w3[0:64, :], d_w3[:])
            nc.sync.dma_start(w3[64:128, :], d_w3[:])
            w4e = cpool.tile([128, 512], dt.bfloat16)
            nc.sync.dma_start(w4e[0:64, :], d_w4e[:])
            nc.sync.dma_start(w4e[64:128, :], d_w4e[:])
            wup = cpool.tile([128, 256], dt.bfloat16)
            nc.sync.dma_start(wup[:].rearrange("p (k c) -> p k c", k=2),
                              d_wup[:].rearrange("(k p) c -> p k c", k=2))
            wlin = cpool.tile([128, 512], dt.bfloat16)
            nc.sync.dma_start(wlin[:].rearrange("p (k c) -> p k c", k=4),
                              d_wlin[:].rearrange("(k p) c -> p k c", k=4))
            wsk = cpool.tile([128, 2560], dt.bfloat16)
            nc.sync.dma_start(wsk[:].rearrange("p (k c) -> p k c", k=20),
                              d_wsk[:].rearrange("(k p) c -> p k c", k=20))
            zk = cpool.tile([128, 128], dt.bfloat16)
            nc.vector.memset(zk[:], 0.0)

            for it in range(ITERS):
                d_table = d_tables[it % 2]
                # ---- Phase 1: linear_up -> table[N_PAD, 512], batched DMA ----
                G8 = 8
                ngrp = (NT_UP + G8 - 1) // G8
                for g in range(ngrp):
                    nts = list(range(g * G8, min((g + 1) * G8, NT_UP)))
                    w = len(nts) * 128
                    slabs = []
                    for comp in range(4):
                        slab = upool.tile([128, G8 * 128], dt.bfloat16, tag="upslab")
                        nc.sync.dma_start(
                            slab[:, :w],
                            d_nfT[comp * 128:(comp + 1) * 128,
                                  nts[0] * 128: nts[0] * 128 + w])
                        slabs.append(slab)
                    stage = wpool.tile([128, G8 * 512], dt.bfloat16, tag="upstage",
                                       bufs=2)
                    for j, nt in enumerate(nts):
                        ps = psA.tile([128, 512], dt.float32, tag="psA", name="psAq")
                        for comp in range(4):
                            nc.tensor.matmul(
                                ps[:, comp * 128:(comp + 1) * 128],
                                slabs[comp][:, j * 128:(j + 1) * 128],
                                wup[:, (0 if comp == 0 else 128):
                                    (128 if comp == 0 else 256)],
                                start=True, stop=True)
                        if j % 2 == 0:
                            nc.scalar.activation(stage[:, j * 512:(j + 1) * 512],
                                                 ps[:], AF.Copy)
                        else:
                            nc.vector.tensor_copy(stage[:, j * 512:(j + 1) * 512],
                                                  ps[:])
                    nc.sync.dma_start(
                        d_table[nts[0] * 128: nts[0] * 128 + w, :].rearrange(
                            "(t p) e -> p t e", p=128),
                        stage[:, :len(nts) * 512].rearrange(
                            "p (t e) -> p t e", e=512))

                # ---- Phase 2: per-block message passing ----
                if PHASES != "p1":
                    osb = bpool.tile([128, BLOCKS * 512], dt.bfloat16,
                                     tag="osb", bufs=2)
                for b in range(BLOCKS if PHASES != "p1" else 0):
                    gb = gpool.tile([128, T_BLK * 512], dt.bfloat16, tag="gather")
                    GCH = 1024
                    for gi in range(E_BLK // GCH):
                        nc.gpsimd.dma_gather(
                            out_ap=gb[:, gi * GCH * 4:(gi + 1) * GCH * 4].rearrange(
                                "p (t e) -> p t e", e=512),
                            in_ap=d_table[:, :],
                            idxs_ap=sidx[:, (b * E_BLK + gi * GCH) // 16:
                                         (b * E_BLK + (gi + 1) * GCH) // 16],
                            num_idxs=GCH, num_idxs_reg=GCH, elem_size=512,
                            queue_num=(b * 2 + gi) % 4,
                        )
                    if PHASES == "p12g":
                        continue
                    efb = wpool.tile([72, E_BLK], dt.bfloat16, tag="efb")
                    nc.sync.dma_start(efb[0:8, :], d_efT[:, b * E_BLK:(b + 1) * E_BLK])
                    nc.sync.dma_start(efb[64:72, :],
                                      d_efT[:, b * E_BLK:(b + 1) * E_BLK])
                    Sall = spool.tile([128, T_BLK * 512], dt.bfloat16, tag="Sall")
                    nc.sync.dma_start(
                        Sall[:].rearrange("p (t c) -> p t c", c=512),
                        d_Sq[b * E_BLK:(b + 1) * E_BLK, :].rearrange(
                            "(t p) c -> p t c", p=128))

                    macc = psB.tile([128, 1024], dt.float32, tag="psB")
                    # zero both macc banks once per block (start=True pends the
                    # whole 2KB zero-region; scatter MMs below all accumulate
                    # with start=False on top of these exact zeros)
                    nc.tensor.matmul(macc[:, 0:512], zk[:], w4e[0:128, :],
                                     start=True, stop=True, skip_group_check=True)
                    nc.tensor.matmul(macc[:, 512:1024], zk[:], w4e[0:128, :],
                                     start=True, stop=True, skip_group_check=True)

                    # run the whole-block radial MLP first so the scalar FIFO
                    # drains silus before the per-tile wt copies queue up
                    h3s = []
                    for chp in range(E_BLK // (2 * MLP_CH)):     # 2 chunks
                        e0 = chp * 2 * MLP_CH
                        h = psH.tile([128, MLP_CH], dt.float32, tag="psH")
                        nc.tensor.matmul(h[0:64, :], w1[0:8, :],
                                         efb[0:8, e0:e0 + MLP_CH],
                                         start=True, stop=True)
                        nc.tensor.matmul(h[64:128, :], w1[64:72, :],
                                         efb[64:72, e0 + MLP_CH:e0 + 2 * MLP_CH],
                                         start=True, stop=True)
                        h1 = wpool.tile([128, MLP_CH], dt.bfloat16, tag="h1")
                        nc.scalar.activation(h1[:], h[:], AF.Silu)
                        h = psH.tile([128, MLP_CH], dt.float32, tag="psH")
                        nc.tensor.matmul(h[0:64, :], w2[0:64, :], h1[0:64, :],
                                         start=True, stop=True)
                        nc.tensor.matmul(h[64:128, :], w2[64:128, :], h1[64:128, :],
                                         start=True, stop=True)
                        h2 = wpool.tile([128, MLP_CH], dt.bfloat16, tag="h2")
                        nc.scalar.activation(h2[:], h[:], AF.Silu)
                        h = psH.tile([128, MLP_CH], dt.float32, tag="psH")
                        nc.tensor.matmul(h[0:64, :], w3[0:64, :], h2[0:64, :],
                                         start=True, stop=True)
                        nc.tensor.matmul(h[64:128, :], w3[64:128, :], h2[64:128, :],
                                         start=True, stop=True)
                        h3 = wpool.tile([128, MLP_CH], dt.bfloat16, tag="h3",
                                        bufs=2)
                        nc.scalar.activation(h3[:], h[:], AF.Silu)
                        h3s.append(h3)

                    for chp, h3 in enumerate(h3s):               # 2 chunks
                        # 8 tiles per chunk; pair (s, s+4) -> concurrent row groups
                        for sp_i in range(MLP_CH // 128):        # 4 pairs
                            wts = []
                            for half in range(2):
                                s = sp_i + 4 * half
                                t_loc = chp * 8 + s
                                wt_ps = psA.tile([128, 512], dt.float32, tag="psA")
                                wts.append((t_loc, wt_ps))
                            # interleave halves: disjoint PE row groups run
                            # concurrently when issued back-to-back
                            for half in range(2):
                                hh = 64 * half
                                nc.tensor.matmul(
                                    wts[half][1][:],
                                    h3[hh:hh + 64,
                                       sp_i * 128:(sp_i + 1) * 128],
                                    w4e[hh:hh + 64, :],
                                    start=True, stop=True,
                                    skip_group_check=True)

                            for t_loc, wt_ps in wts:
                                t = b * T_BLK + t_loc
                                G = gb[:, t_loc * 512:(t_loc + 1) * 512]
                                # scalar: copy [ws1|wv1] -> sbuf bf16
                                wt = wpool.tile([128, 256], dt.bfloat16, tag="wt")
                                nc.scalar.activation(wt[:], wt_ps[:, 0:256], AF.Copy)
                                # msg layout: [m_s1 | u | m_v2(3x128) | b(3x128)]
                                msg = mpool.tile([128, 1024], dt.bfloat16, tag="msg")
                                # A+C fused: [m_s1|u] = G_s . [ws1|wv1]
                                nc.vector.tensor_tensor(
                                    msg[:, 0:256].rearrange(
                                        "p (k c) -> p k c", k=2),
                                    G[:, 0:128].unsqueeze(1).broadcast_to(
                                        (128, 2, 128)),
                                    wt[:].rearrange("p (k c) -> p k c", k=2),
                                    OP.mult)
                                # B+D fused: [m_v2|b] = G_v . [wv2|ws2] bcast
                                nc.vector.tensor_tensor(
                                    msg[:, 256:1024].rearrange(
                                        "p (k i c) -> p k i c", k=2, i=3),
                                    G[:, 128:512].rearrange(
                                        "p (i c) -> p i c", i=3
                                    ).unsqueeze(1).broadcast_to(
                                        (128, 2, 3, 128)),
                                    wt_ps[:, 256:512].rearrange(
                                        "p (k c) -> p k c", k=2
                                    ).unsqueeze(2).broadcast_to(
                                        (128, 2, 3, 128)),
                                    OP.mult)

                                # msg-stationary scatter into macc [feat, node]
                                Sy = [Sall[:, t_loc * 512 + 128 * i:
                                           t_loc * 512 + 128 * (i + 1)]
                                      for i in range(4)]
                                srcs = [msg[:, 0:128], msg[:, 256:384],
                                        msg[:, 384:512], msg[:, 512:640]]
                                for k in range(4):   # m_s1, m_v2_i <- Sy0
                                    nc.tensor.matmul(
                                        macc[:, k * 128:(k + 1) * 128],
                                        srcs[k], Sy[0],
                                        start=False, stop=False,
                                        skip_group_check=True)
                                for i in range(3):   # m_v1_i = u^T Sy_{1+i}
                                    nc.tensor.matmul(
                                        macc[:, 512 + i * 128:640 + i * 128],
                                        msg[:, 128:256], Sy[1 + i],
                                        start=False, stop=False,
                                        skip_group_check=True)
                                for i in range(3):   # m_s2 += b_i^T Sy_{1+i}
                                    nc.tensor.matmul(
                                        macc[:, 896:1024],
                                        msg[:, 640 + i * 128:768 + i * 128],
                                        Sy[1 + i],
                                        start=False, stop=False,
                                        skip_group_check=True)

                    # ---- per-block: macc -> sbuf, mid linear ----
                    # macc layout: [s1 | v2_0 v2_1 v2_2 | v1_0 v1_1 v1_2 | s2]
                    MT = bpool.tile([128, 1024], dt.bfloat16, tag="MT")
                    nc.scalar.activation(MT[:], macc[:], AF.Copy)
                    pm = psA.tile([128, 512], dt.float32, tag="psA", name="pmq")
                    # out_s = wlin0_c0^T s1 + wlin0_c1^T s2
                    nc.tensor.matmul(pm[:, 0:128], wlin[:, 0:128], MT[:, 0:128],
                                     start=True, stop=False, skip_group_check=True)
                    nc.tensor.matmul(pm[:, 0:128], wlin[:, 128:256],
                                     MT[:, 896:1024],
                                     start=False, stop=True, skip_group_check=True)
                    # out_v_i = wlin1_c0^T v1_i + wlin1_c1^T v2_i
                    for i in range(3):
                        nc.tensor.matmul(pm[:, 128 * (1 + i):128 * (2 + i)],
                                         wlin[:, 256:384],
                                         MT[:, 512 + i * 128:640 + i * 128],
                                         start=True, stop=False,
                                         skip_group_check=True)
                        nc.tensor.matmul(pm[:, 128 * (1 + i):128 * (2 + i)],
                                         wlin[:, 384:512],
                                         MT[:, 128 + i * 128:256 + i * 128],
                                         start=False, stop=True,
                                         skip_group_check=True)
                    # osb layout [c, x, block, node]: flat 256-col slices for
                    # the phase-3 Z multiplies
                    for x in range(4):
                        nc.scalar.activation(
                            osb[:, x * LNPAD + b * 128: x * LNPAD + (b + 1) * 128],
                            pm[:, x * 128:(x + 1) * 128], AF.Copy)

                # ---- Phase 3: skip_tp (5 node groups of 256 = 2 blocks) ----
                HNODES = LNPAD // 5
                for hf in range(5 if PHASES == "full" else 0):
                    for x in range(4):
                        po = psH.tile([128, MLP_CH], dt.float32,
                                      tag="psH", name="poq")[:, 0:HNODES]
                        osb_x = osb[:, x * LNPAD + hf * HNODES:
                                    x * LNPAD + (hf + 1) * HNODES]
                        for a in range(A):
                            Z = wpool.tile([128, HNODES], dt.bfloat16, tag="Z")
                            ar = arep[:, a * LNPAD + hf * HNODES:
                                      a * LNPAD + (hf + 1) * HNODES]
                            eng = nc.vector if (a % 2 == 0) else nc.gpsimd
                            eng.tensor_tensor(Z[:], osb_x, ar,
                                              mybir.AluOpType.mult)
                            wchunk = wsk[:, ((0 if x == 0 else 10) + a) * 128:
                                         ((0 if x == 0 else 10) + a) * 128 + 128]
                            nc.tensor.matmul(po[:], wchunk, Z[:],
                                             start=(a == 0), stop=(a == A - 1),
                                             skip_group_check=True)
                        oout = wpool.tile([128, HNODES], dt.float32, tag="oout")
                        nc.scalar.activation(oout[:], po[:], AF.Copy)
                        nc.sync.dma_start(
                            d_out[x * 128:(x + 1) * 128,
                                  hf * HNODES:(hf + 1) * HNODES], oout[:])

    nc.compile()
    return nc


def kernel(**inputs):
    in_maps = _prep_host(**inputs)
    if "nc" not in _CACHE:
        _CACHE["nc"] = _build_nc()
    nc = _CACHE["nc"]
    from concourse.bass_utils import run_bass_kernel_spmd
    res = run_bass_kernel_spmd(nc, in_maps, core_ids=list(range(NCORES)))
    return _assemble_output(res.results)
